# revision 18
# baseline (speedup 1.0000x reference)
"""Pointer-network decoder (LSTM + additive attention + greedy pointer) on 8 trn2 cores.

Data-parallel over batch B=256 -> 32 rows per core. Each core runs the full
128-step sequential decode for its batch shard; no collectives.

Per-step device pipeline (per core, b=32, H=512 as 4 chunks of 128):
  gates   = Wi.T@xT + Wh.T@hT      (PE, fp32, weights stationary)  -> PSUM [128,(mc,b)]
  sigmoid (ACT, bias=b_i+b_h per partition)                        -> sigT
  c,h_i   updates (DVE tensor_tensor; ACT tanh)                    -> cT, h_iT
  qT      = Wha.T@h_iT             (PE fp32)                       -> qT
  th      = ctx_projT + qT (DVE tensor_scalar broadcast-add)       -> th tiles
  th      = tanh(th) in-place      (ACT, big free dims)
  scoresT = v . th (PE: th stationary fp32, v moving, N=1)         -> PSUM [s,b]
  softmax (transpose -> mask -> max -> exp(accum) -> recip)        -> alpha
  argmax  (is_ge + iota trick), mask update, x gather (indirect DMA)
  attT    = context.T @ alphaT (PE bf16, context stationary, FWL)
  h       = tanh(Wout.T@[attT;h_iT] + bout)  (PE fp32 + ACT)
alpha/idx scattered to DRAM by row-index indirect DMA each step.
"""

import numpy as np
import ml_dtypes

B, S, H = 256, 128, 512
NCORES = 8
BS = B // NCORES  # 32 batch rows per core
HC = H // 128     # 4 h-chunks
F32MAX_BIG = 1000.0


def _prep_w_km(w, k, m):
    # (K, M) -> sbuf layout [128, (kc, M)]: col block kc*M + j = w[kc*128+p, j]
    kc = k // 128
    return np.ascontiguousarray(
        w.reshape(kc, 128, m).transpose(1, 0, 2).reshape(128, kc * m)
    ).astype(np.float32)


def _prep_T(x):
    # (BS, H) -> [128, (hc, BS)]: col hc*BS+b = x.T[hc*128+p, b]
    return np.ascontiguousarray(
        x.T.reshape(HC, 128, BS).transpose(1, 0, 2).reshape(128, HC * BS)
    ).astype(np.float32)


def build_bass(n_steps=S):
    import os
    import concourse.bass as bass
    import concourse.bacc as bacc
    import concourse.tile as tile
    from concourse import mybir

    ablate = set(os.environ.get("KERNEL_ABLATE", "").split(","))

    f32 = mybir.dt.float32
    bf16 = mybir.dt.bfloat16
    i32 = mybir.dt.int32
    AF = mybir.ActivationFunctionType
    Alu = mybir.AluOpType
    X = mybir.AxisListType.X

    nc = bacc.Bacc()

    # ---- DRAM I/O -------------------------------------------------------
    d_Wi = nc.declare_dram_parameter("Wi", [128, 4 * 2048], f32, isOutput=False)
    d_Wh = nc.declare_dram_parameter("Wh", [128, 4 * 2048], f32, isOutput=False)
    d_Wha = nc.declare_dram_parameter("Wha", [128, 4 * 512], f32, isOutput=False)
    d_Wout = nc.declare_dram_parameter("Wout", [128, 8 * 512], f32, isOutput=False)
    d_Wctx = nc.declare_dram_parameter("Wctx", [128, 4 * 512], f32, isOutput=False)
    d_bsum = nc.declare_dram_parameter("bsum", [128, 16], f32, isOutput=False)
    d_bout = nc.declare_dram_parameter("bout", [128, 4], f32, isOutput=False)
    d_v = nc.declare_dram_parameter("v4", [128, 4], f32, isOutput=False)
    d_biginv = nc.declare_dram_parameter("biginv", [BS, 128], f32, isOutput=False)
    d_ident = nc.declare_dram_parameter("ident", [128, 128], f32, isOutput=False)
    d_row0 = nc.declare_dram_parameter("row0f", [BS, 1], f32, isOutput=False)
    d_row0b = nc.declare_dram_parameter("row0b", [BS, 1], f32, isOutput=False)
    d_hT0 = nc.declare_dram_parameter("hT0", [128, HC * BS], f32, isOutput=False)
    d_cT0 = nc.declare_dram_parameter("cT0", [128, HC * BS], f32, isOutput=False)
    d_xT0 = nc.declare_dram_parameter("xT0", [128, HC * BS], f32, isOutput=False)
    d_ctxT = nc.declare_dram_parameter("ctxT", [BS, H, S], f32, isOutput=False)
    d_ctxbf = nc.declare_dram_parameter("ctxbf", [BS * S, H], bf16, isOutput=False)
    d_emb = nc.declare_dram_parameter("emb", [BS * S, H], f32, isOutput=False)

    d_oal = nc.declare_dram_parameter("out_al", [BS * S, S], f32, isOutput=True)
    d_oidx = nc.declare_dram_parameter("out_idx", [BS * S, 1], f32, isOutput=True)

    with tile.TileContext(nc) as tc:
        import contextlib

        stack = contextlib.ExitStack()
        singles = stack.enter_context(tc.tile_pool(name="singles", bufs=1))
        work = stack.enter_context(tc.tile_pool(name="work", bufs=2))
        thpool = stack.enter_context(tc.tile_pool(name="thp", bufs=2))
        stage = stack.enter_context(tc.tile_pool(name="stage", bufs=6))
        psum = stack.enter_context(tc.tile_pool(name="ps", bufs=1, space="PSUM"))

        # ---- persistent SBUF tensors -----------------------------------
        Wi_sb = singles.tile([128, 8192], f32, name="Wi_sb")
        Wh_sb = singles.tile([128, 8192], f32, name="Wh_sb")
        Wha_sb = singles.tile([128, 2048], f32, name="Wha_sb")
        Wout_sb = singles.tile([128, 4096], f32, name="Wout_sb")
        bsum_sb = singles.tile([128, 16], f32, name="bsum_sb")
        bout_sb = singles.tile([128, 4], f32, name="bout_sb")
        v_sb = singles.tile([128, 4], f32, name="v_sb")
        biginv_sb = singles.tile([BS, 128], f32, name="biginv_sb")
        ident_sb = singles.tile([128, 128], f32, name="ident_sb")
        row0b_sb = singles.tile([BS, 1], f32, name="row0b_sb")
        hT = singles.tile([128, HC * BS], f32, name="hT")
        cT = singles.tile([128, HC * BS], f32, name="cT")
        xT = singles.tile([128, HC * BS], f32, name="xT")
        hiT = singles.tile([128, HC * BS], f32, name="hiT")
        sigT = singles.tile([128, 16 * BS], f32, name="sigT")
        qT = singles.tile([128, HC * BS], f32, name="qT")
        attT_sb = singles.tile([128, HC * BS], f32, name="attT_sb")
        maskneg = singles.tile([BS, 128], f32, name="maskneg")
        rowf = singles.tile([BS, 1], f32, name="rowf")
        cpT = [
            singles.tile([128, BS * S], f32, name=f"cpT{hc}") for hc in range(HC)
        ]

        for sb, dr in [
            (Wi_sb, d_Wi), (Wh_sb, d_Wh), (Wha_sb, d_Wha), (Wout_sb, d_Wout),
            (bsum_sb, d_bsum), (bout_sb, d_bout), (v_sb, d_v),
            (biginv_sb, d_biginv), (ident_sb, d_ident),
            (row0b_sb, d_row0b), (hT, d_hT0), (cT, d_cT0), (xT, d_xT0),
            (rowf, d_row0),
        ]:
            nc.sync.dma_start(out=sb[:, :], in_=dr[:, :])
        nc.vector.memset(maskneg[:, :], 0.0)

        # ---- init: ctx_projT[hc][:, b*S + s] = (context[b] @ Wctx).T ----
        with tc.tile_pool(name="initp", bufs=2) as initp:
            Wctx_sb = initp.tile([128, 2048], f32, name="Wctx_sb", bufs=1)
            nc.sync.dma_start(out=Wctx_sb[:, :], in_=d_Wctx[:, :])
            for b in range(BS):
                cstage = initp.tile([128, HC, S], f32, name="cstage", tag="cstage", bufs=1)
                nc.sync.dma_start(
                    out=cstage[:, :, :],
                    in_=d_ctxT[b].rearrange("(k p) s -> p k s", p=128),
                )
                for mc in range(HC):
                    cp_ps = psum.tile([128, S], f32, name="cp_ps", tag="gates")
                    for kc in range(HC):
                        nc.tensor.matmul(
                            out=cp_ps[:, :],
                            lhsT=Wctx_sb[:, kc * 512 + mc * 128 : kc * 512 + (mc + 1) * 128],
                            rhs=cstage[:, kc, :],
                            start=(kc == 0),
                            stop=(kc == HC - 1),
                        )
                    nc.vector.tensor_copy(
                        out=cpT[mc][:, b * S : (b + 1) * S], in_=cp_ps[:, :]
                    )

        # ---- the sequential decode loop --------------------------------
        with tc.For_i(0, n_steps, 1, hint_engines=(mybir.EngineType.PE,)) as _t:
            # PSUM tiles for this step
            gates_ps = psum.tile([128, 16 * BS], f32, name="gates_ps", tag="gates")
            q_ps = psum.tile([128, HC * BS], f32, name="q_ps", tag="q")
            scores_ps = psum.tile([128, BS], f32, name="scores_ps", tag="scores")
            scbm_ps = psum.tile([BS, 128], f32, name="scbm_ps", tag="scbm")
            alT_ps = psum.tile([128, BS], f32, name="alT_ps", tag="alT")
            attT_ps = psum.tile([128, HC * BS], f32, name="attT_ps", tag="attT")
            hnew_ps = psum.tile([128, HC * BS], f32, name="hnew_ps", tag="hnew")
            xT_ps = psum.tile([128, HC * BS], f32, name="xT_ps", tag="xTp")

            # 1. gates
            for mc in range(16 if "nope" not in ablate else 0):
                n8 = 0
                for kc in range(HC):
                    for W_sb in (Wi_sb, Wh_sb):
                        rhs = xT if W_sb is Wi_sb else hT
                        nc.tensor.matmul(
                            out=gates_ps[:, mc * BS : (mc + 1) * BS],
                            lhsT=W_sb[:, kc * 2048 + mc * 128 : kc * 2048 + (mc + 1) * 128],
                            rhs=rhs[:, kc * BS : (kc + 1) * BS],
                            start=(n8 == 0),
                            stop=(n8 == 7),
                        )
                        n8 += 1
            # 2. sigmoid on all 4H (reference applies sigmoid to all gates)
            for mc in range(16):
                nc.scalar.activation(
                    out=sigT[:, mc * BS : (mc + 1) * BS],
                    in_=gates_ps[:, mc * BS : (mc + 1) * BS],
                    func=AF.Sigmoid,
                    bias=bsum_sb[:, mc : mc + 1],
                )
            # 3. c = f*c + i*g ; h_i = o*tanh(c)   (chunk hc slices)
            for hc in range(HC):
                sl = slice(hc * BS, (hc + 1) * BS)
                # gate chunk layout: i_=mc 0..3, f_=4..7, g_=8..11, o_=12..15
                i_s = sigT[:, (0 + hc) * BS : (0 + hc + 1) * BS]
                f_s = sigT[:, (4 + hc) * BS : (4 + hc + 1) * BS]
                g_s = sigT[:, (8 + hc) * BS : (8 + hc + 1) * BS]
                o_s = sigT[:, (12 + hc) * BS : (12 + hc + 1) * BS]
                t1 = work.tile([128, BS], f32, name="t1", tag="t1")
                t2 = work.tile([128, BS], f32, name="t2", tag="t2")
                nc.vector.tensor_tensor(out=t1[:, :], in0=f_s, in1=cT[:, sl], op=Alu.mult)
                nc.vector.tensor_tensor(out=t2[:, :], in0=i_s, in1=g_s, op=Alu.mult)
                nc.vector.tensor_tensor(out=cT[:, sl], in0=t1[:, :], in1=t2[:, :], op=Alu.add)
                t3 = work.tile([128, BS], f32, name="t3", tag="t3")
                nc.scalar.activation(out=t3[:, :], in_=cT[:, sl], func=AF.Tanh)
                nc.vector.tensor_tensor(out=hiT[:, sl], in0=o_s, in1=t3[:, :], op=Alu.mult)
            # 4. qT = Wha.T @ hiT
            for mc in range(HC if "nope" not in ablate else 0):
                for kc in range(HC):
                    nc.tensor.matmul(
                        out=q_ps[:, mc * BS : (mc + 1) * BS],
                        lhsT=Wha_sb[:, kc * 512 + mc * 128 : kc * 512 + (mc + 1) * 128],
                        rhs=hiT[:, kc * BS : (kc + 1) * BS],
                        start=(kc == 0),
                        stop=(kc == HC - 1),
                    )
            if "nope" not in ablate:
                nc.vector.tensor_copy(out=qT[:, :], in_=q_ps[:, :])
            else:
                nc.vector.memset(q_ps[:, :], 0.0)
                nc.vector.memset(gates_ps[:, :], 0.0)
                nc.vector.memset(scores_ps[:, :], 0.0)
                nc.vector.memset(attT_ps[:, :], 0.0)
                nc.vector.memset(hnew_ps[:, :], 0.0)
                nc.vector.tensor_copy(out=qT[:, :], in_=q_ps[:, :])

            # 5. attention: th = tanh(ctx_projT + qT) ; scoresT = v . th
            # One th tile per group of GB batch rows, containing all HC chunks,
            # so each b's 4-matmul PSUM accumulation group is contiguous
            # (only one pending group per PSUM bank is allowed).
            GB = 4  # b-rows per th tile
            for g in range(BS // GB):
                th = thpool.tile([128, HC * GB * S], f32, name="th", tag="th")
                for hc in range(HC if "noadds" not in ablate else 0):
                    for j in range(GB):
                        b = g * GB + j
                        nc.vector.tensor_scalar_add(
                            out=th[:, (hc * GB + j) * S : (hc * GB + j + 1) * S],
                            in0=cpT[hc][:, b * S : (b + 1) * S],
                            scalar1=qT[:, hc * BS + b : hc * BS + b + 1],
                        )
                if "notanh" not in ablate:
                    nc.scalar.activation(out=th[:, :], in_=th[:, :], func=AF.Tanh)
                for j in range(GB if "nope" not in ablate else 0):
                    b = g * GB + j
                    for hc in range(HC):
                        nc.tensor.matmul(
                            out=scores_ps[:, b : b + 1],
                            lhsT=th[:, (hc * GB + j) * S : (hc * GB + j + 1) * S],
                            rhs=v_sb[:, hc : hc + 1],
                            start=(hc == 0),
                            stop=(hc == HC - 1),
                        )
            # 6. softmax over s (b-major layout)
            sc_sT = work.tile([128, BS], f32, name="sc_sT", tag="sc_sT")
            nc.vector.tensor_copy(out=sc_sT[:, :], in_=scores_ps[:, :])
            nc.tensor.transpose(out=scbm_ps[:, :], in_=sc_sT[:, :], identity=ident_sb[:, :])
            msc = work.tile([BS, 128], f32, name="msc", tag="msc")
            nc.vector.tensor_tensor(out=msc[:, :], in0=scbm_ps[:, :], in1=maskneg[:, :], op=Alu.add)
            mx = work.tile([BS, 1], f32, name="mx", tag="mx")
            nc.vector.reduce_max(out=mx[:, :], in_=msc[:, :], axis=X)
            mxn = work.tile([BS, 1], f32, name="mxn", tag="mxn")
            nc.vector.tensor_scalar_mul(out=mxn[:, :], in0=mx[:, :], scalar1=-1.0)
            e_t = work.tile([BS, 128], f32, name="e_t", tag="e_t")
            sm = work.tile([BS, 1], f32, name="sm", tag="sm")
            nc.scalar.activation(
                out=e_t[:, :], in_=msc[:, :], func=AF.Exp,
                bias=mxn[:, 0:1], scale=1.0, accum_out=sm[:, 0:1],
            )
            rcp = work.tile([BS, 1], f32, name="rcp", tag="rcp")
            nc.vector.reciprocal(out=rcp[:, :], in_=sm[:, :])
            alpha = work.tile([BS, 128], f32, name="alpha", tag="alpha")
            nc.vector.tensor_scalar_mul(out=alpha[:, :], in0=e_t[:, :], scalar1=rcp[:, 0:1])
            # 7. write alpha rows to DRAM  (row = b*S + t)
            rowi = work.tile([BS, 1], i32, name="rowi", tag="rowi")
            nc.vector.tensor_copy(out=rowi[:, :], in_=rowf[:, :])
            if "noind" not in ablate:
                nc.gpsimd.indirect_dma_start(
                    out=d_oal[:, :],
                    out_offset=bass.IndirectOffsetOnAxis(ap=rowi[:, :1], axis=0),
                    in_=alpha[:, :],
                    in_offset=None,
                )
            else:
                nc.sync.dma_start(
                    out=d_oal.rearrange("(b s) h -> b s h", s=S)[:, 0, :],
                    in_=alpha[:, :],
                )
            # 8. argmax (first max index) of alpha (alpha is exactly 0 at
            # masked positions, so masking is a no-op). mxc = biginv at the
            # first-max position; index = BIG - mxc (host converts out_idx).
            mx2 = work.tile([BS, 1], f32, name="mx2", tag="mx2")
            nc.vector.reduce_max(out=mx2[:, :], in_=alpha[:, :], axis=X)
            eqv = work.tile([BS, 128], f32, name="eqv", tag="eqv")
            nc.vector.tensor_scalar(
                out=eqv[:, :], in0=alpha[:, :], scalar1=mx2[:, 0:1], scalar2=None,
                op0=Alu.is_ge,
            )
            cnd = work.tile([BS, 128], f32, name="cnd", tag="cnd")
            nc.vector.tensor_tensor(out=cnd[:, :], in0=eqv[:, :], in1=biginv_sb[:, :], op=Alu.mult)
            mxc = work.tile([BS, 1], f32, name="mxc", tag="mxc")
            nc.vector.reduce_max(out=mxc[:, :], in_=cnd[:, :], axis=X)
            if "noind" not in ablate:
                nc.gpsimd.indirect_dma_start(
                    out=d_oidx[:, :],
                    out_offset=bass.IndirectOffsetOnAxis(ap=rowi[:, :1], axis=0),
                    in_=mxc[:, :],
                    in_offset=None,
                )
            # 9. pointer one-hot (biginv values are unique); maskneg -= oh*1e9
            oh = work.tile([BS, 128], f32, name="oh", tag="oh")
            nc.vector.tensor_scalar(
                out=oh[:, :], in0=biginv_sb[:, :], scalar1=mxc[:, 0:1], scalar2=None,
                op0=Alu.is_equal,
            )
            t4 = work.tile([BS, 128], f32, name="t4", tag="t4")
            nc.vector.tensor_scalar_mul(out=t4[:, :], in0=oh[:, :], scalar1=1e9)
            nc.vector.tensor_tensor(out=maskneg[:, :], in0=maskneg[:, :], in1=t4[:, :], op=Alu.subtract)
            # 10. x gather: rows b*S + idx
            gxf = work.tile([BS, 1], f32, name="gxf", tag="gxf")
            nc.vector.tensor_tensor(out=gxf[:, :], in0=row0b_sb[:, :], in1=mxc[:, :], op=Alu.subtract)
            gxi = work.tile([BS, 1], i32, name="gxi", tag="gxi")
            nc.vector.tensor_copy(out=gxi[:, :], in_=gxf[:, :])
            xrow = work.tile([BS, H], f32, name="xrow", tag="xrow")
            if "noind" not in ablate:
                nc.gpsimd.indirect_dma_start(
                    out=xrow[:, :],
                    out_offset=None,
                    in_=d_emb[:, :],
                    in_offset=bass.IndirectOffsetOnAxis(ap=gxi[:, :1], axis=0),
                )
            else:
                nc.sync.dma_start(
                    out=xrow[:, :],
                    in_=d_emb.rearrange("(b s) h -> b s h", s=S)[:, 0, :],
                )
            for hc in range(HC):
                nc.tensor.transpose(
                    out=xT_ps[:, hc * BS : (hc + 1) * BS],
                    in_=xrow[:, hc * 128 : (hc + 1) * 128],
                    identity=ident_sb[:BS, :BS],
                )
            nc.vector.tensor_copy(out=xT[:, :], in_=xT_ps[:, :])
            # 11. att: attT[:, hc*BS+b] = context[b,:,hc].T @ alphaT[:,b]
            nc.tensor.transpose(out=alT_ps[:, :], in_=alpha[:, :], identity=ident_sb[:BS, :BS])
            alT_bf = work.tile([128, BS], bf16, name="alT_bf", tag="alT_bf")
            nc.vector.tensor_copy(out=alT_bf[:, :], in_=alT_ps[:, :])
            for b in range(BS):
                cstg = stage.tile([128, H], bf16, name="cstg", tag="cstg")
                if "noctxdma" not in ablate:
                    nc.sync.dma_start(out=cstg[:, :], in_=d_ctxbf[b * S : (b + 1) * S, :])
                for hc in range(HC if "nope" not in ablate else 0):
                    nc.tensor.matmul(
                        out=attT_ps[:, hc * BS + b : hc * BS + b + 1],
                        lhsT=cstg[:, hc * 128 : (hc + 1) * 128],
                        rhs=alT_bf[:, b : b + 1],
                        start=True,
                        stop=True,
                    )
            nc.vector.tensor_copy(out=attT_sb[:, :], in_=attT_ps[:, :])
            # 12. h = tanh(Wout.T @ [attT; hiT] + bout)
            for mc in range(HC if "nope" not in ablate else 0):
                for kc in range(8):
                    rhs = (
                        attT_sb[:, kc * BS : (kc + 1) * BS]
                        if kc < HC
                        else hiT[:, (kc - HC) * BS : (kc - HC + 1) * BS]
                    )
                    nc.tensor.matmul(
                        out=hnew_ps[:, mc * BS : (mc + 1) * BS],
                        lhsT=Wout_sb[:, kc * 512 + mc * 128 : kc * 512 + (mc + 1) * 128],
                        rhs=rhs,
                        start=(kc == 0),
                        stop=(kc == 7),
                    )
            for mc in range(HC):
                nc.scalar.activation(
                    out=hT[:, mc * BS : (mc + 1) * BS],
                    in_=hnew_ps[:, mc * BS : (mc + 1) * BS],
                    func=AF.Tanh,
                    bias=bout_sb[:, mc : mc + 1],
                )
            # 13. advance output row pointer
            nc.vector.tensor_scalar_add(out=rowf[:, :], in0=rowf[:, :], scalar1=1.0)

        stack.close()
    nc.compile()
    return nc


def _core_inputs(inputs, core):
    sl = slice(core * BS, (core + 1) * BS)
    ctx = np.asarray(inputs["context"], np.float32)[sl]
    emb = np.asarray(inputs["embeded_inputs"], np.float32)[sl]
    m = {
        "Wi": _prep_w_km(np.asarray(inputs["Wi"], np.float32), H, 4 * H),
        "Wh": _prep_w_km(np.asarray(inputs["Wh"], np.float32), H, 4 * H),
        "Wha": _prep_w_km(np.asarray(inputs["Wha"], np.float32), H, H),
        "Wout": _prep_w_km(np.asarray(inputs["Wout"], np.float32), 2 * H, H),
        "Wctx": _prep_w_km(np.asarray(inputs["Wctx"], np.float32), H, H),
        "bsum": np.ascontiguousarray(
            (np.asarray(inputs["bi"]) + np.asarray(inputs["bh"]))
            .astype(np.float32).reshape(16, 128).T
        ),
        "bout": np.ascontiguousarray(
            np.asarray(inputs["bout"], np.float32).reshape(4, 128).T
        ),
        "v4": np.ascontiguousarray(
            np.asarray(inputs["v"], np.float32).reshape(4, 128).T
        ),
        "biginv": F32MAX_BIG - np.tile(np.arange(128, dtype=np.float32), (BS, 1)),
        "ident": np.eye(128, dtype=np.float32),
        "row0f": (np.arange(BS, dtype=np.float32) * S).reshape(BS, 1),
        "row0b": (F32MAX_BIG + np.arange(BS, dtype=np.float32) * S).reshape(BS, 1),
        "hT0": _prep_T(np.asarray(inputs["h0"], np.float32)[sl]),
        "cT0": _prep_T(np.asarray(inputs["c0"], np.float32)[sl]),
        "xT0": _prep_T(np.asarray(inputs["decoder_input"], np.float32)[sl]),
        "ctxT": np.ascontiguousarray(ctx.transpose(0, 2, 1)),
        "ctxbf": np.ascontiguousarray(ctx.reshape(BS * S, H)).astype(
            ml_dtypes.bfloat16
        ),
        "emb": np.ascontiguousarray(emb.reshape(BS * S, H)),
    }
    return m


_CACHED = {}


def run(inputs, n_steps=S, trace=False):
    from concourse.bass_utils import run_bass_kernel_spmd

    key = n_steps
    if key not in _CACHED:
        _CACHED[key] = build_bass(n_steps)
    nc = _CACHED[key]
    in_maps = [_core_inputs(inputs, c) for c in range(NCORES)]
    res = run_bass_kernel_spmd(nc, in_maps, list(range(NCORES)), trace=trace)
    outs = []
    ptrs = []
    for c in range(NCORES):
        al = np.asarray(res.results[c]["out_al"]).reshape(BS, S, S)
        ix = F32MAX_BIG - np.asarray(res.results[c]["out_idx"]).reshape(BS, S)
        outs.append(al)
        ptrs.append(ix)
    outputs = np.concatenate(outs, axis=0).astype(np.float32)
    pointers = np.concatenate(ptrs, axis=0).astype(np.int32)
    return (outputs, pointers), res


def kernel(**inputs):
    (outputs, pointers), _ = run(inputs)
    return outputs, pointers
